# revision 1
# baseline (speedup 1.0000x reference)
"""Trainium2 Bass kernel for multi-head attention with RoPE (B=2, S=2048,
D=2048, H=16), distributed over 8 NeuronCores with head tensor-parallelism
and an AllToAll to switch to token-parallelism for the output projection.

kernel(**inputs) takes the full unsharded inputs (as produced by the
reference setup_inputs) and returns the full [2, 2048, 2048] f32 output.
"""
import numpy as np
import ml_dtypes
from concourse import bass, bacc, tile, mybir
from concourse.bass_utils import run_bass_kernel_spmd

bf16 = ml_dtypes.bfloat16
BF16 = mybir.dt.bfloat16
F32 = mybir.dt.float32
AF = mybir.ActivationFunctionType
OP = mybir.AluOpType

B, S, D, H = 2, 2048, 2048, 16
HD = 128                 # head dim
NCORES = 8
HL = H // NCORES         # heads per core = 2
EL = HL * HD             # local projection width = 256
T = B * S                # 4096 flattened tokens
TCH = 512                # token chunk for QKV phase
NTCH = T // TCH          # 8
NKT = S // 128           # 16 key tiles per batch
NQC = S // 512           # 4 query chunks per batch
NDT = D // 128           # 16 contraction tiles
TL = T // NCORES         # 512 tokens per core after AllToAll
SCALE = float(1.0 / np.sqrt(128.0))

_CACHE = {}
_DEBUG = False


def _build():
    nc = bacc.Bacc("TRN2", target_bir_lowering=False, num_devices=NCORES)
    dbg = {}
    if _DEBUG:
        dbg["qT0"] = nc.dram_tensor("dbg_qT0", [128, T], BF16, kind="ExternalOutput")
        dbg["kT0"] = nc.dram_tensor("dbg_kT0", [128, T], BF16, kind="ExternalOutput")
        dbg["vn0"] = nc.dram_tensor("dbg_vn0", [128, T], BF16, kind="ExternalOutput")
        dbg["xt0"] = nc.dram_tensor("dbg_xt0", [128, TCH], BF16, kind="ExternalOutput")
        dbg["E0"] = nc.dram_tensor("dbg_E0", [128, S], BF16, kind="ExternalOutput")
        dbg["on0"] = nc.dram_tensor("dbg_on0", [128, 512], BF16, kind="ExternalOutput")
        dbg["rec0"] = nc.dram_tensor("dbg_rec0", [128, 512], F32, kind="ExternalOutput")
        dbg["ot0"] = nc.dram_tensor("dbg_ot0", [128, TL], BF16, kind="ExternalOutput")
        dbg["a2ain"] = nc.dram_tensor("dbg_a2ain", [NCORES, EL, TL], BF16, kind="ExternalOutput")
        dbg["a2aout"] = nc.dram_tensor("dbg_a2aout", [NCORES, EL, TL], BF16, kind="ExternalOutput")
        dbg["y0"] = nc.dram_tensor("dbg_y0", [128, 512], F32, kind="ExternalOutput")

    x = nc.dram_tensor("x", [T, D], F32, kind="ExternalInput")
    wq_t = nc.dram_tensor("wq_t", [D, EL], BF16, kind="ExternalInput")
    wk_t = nc.dram_tensor("wk_t", [D, EL], BF16, kind="ExternalInput")
    wv_t = nc.dram_tensor("wv_t", [D, EL], BF16, kind="ExternalInput")
    wo_t = nc.dram_tensor("wo_t", [D, D], BF16, kind="ExternalInput")
    cos_t = nc.dram_tensor("cos_t", [HD, S], BF16, kind="ExternalInput")
    sin_m = nc.dram_tensor("sin_m", [HD, S], BF16, kind="ExternalInput")
    mask_t = nc.dram_tensor("mask_t", [128, B * NKT], F32, kind="ExternalInput")
    out = nc.dram_tensor("out", [TL, D], F32, kind="ExternalOutput")

    ident_dram = nc.inline_tensor(np.eye(128, dtype=bf16), name="ident")
    ones_dram = nc.inline_tensor(np.ones((128, 128), dtype=bf16), name="ones")

    with tile.TileContext(nc) as tc:
        with (
            # ---------- persistent pools (whole kernel) ----------
            tc.tile_pool(name="dram", bufs=1, space="DRAM") as dram,
            tc.tile_pool(name="consts", bufs=1) as consts,
            tc.tile_pool(name="qkv_keep", bufs=1) as keep,
        ):
            xbf = [dram.tile([1024, D], BF16, tag=f"xbf{c}", name=f"xbf{c}")
                   for c in range(T // 1024)]
            a2a_in = [dram.tile([NCORES, HD, TL], BF16, tag=f"a2a_in{h}", name=f"a2a_in{h}")
                      for h in range(HL)]
            a2a_out = [dram.tile([NCORES, HD, TL], BF16, tag=f"a2a_out{h}", name=f"a2a_out{h}")
                       for h in range(HL)]

            ident_sb = consts.tile([128, 128], BF16, tag="ident", name="ident_sb")
            nc.sync.dma_start(ident_sb[:], ident_dram[:])
            ones_sb = consts.tile([128, 128], BF16, tag="ones", name="ones_sb")
            nc.sync.dma_start(ones_sb[:], ones_dram[:])
            mask_sb = consts.tile([128, B * NKT], F32, tag="mask", name="mask_sb")
            nc.sync.dma_start(mask_sb[:], mask_t[:])

            # persistent per-head tensors ([hd, t] layouts; v natural packed
            # as 32 [t=128, hd=128] tiles along free dim)
            qT = [keep.tile([128, T], BF16, tag=f"qT{h}", name=f"qT{h}") for h in range(HL)]
            kT = [keep.tile([128, T], BF16, tag=f"kT{h}", name=f"kT{h}") for h in range(HL)]
            vnat = [keep.tile([128, T], BF16, tag=f"vn{h}", name=f"vn{h}") for h in range(HL)]

            # ---------- phase A+B: x prep + QKV projections + RoPE ----------
            with (
                tc.tile_pool(name="wsb", bufs=1) as wpool,
                tc.tile_pool(name="tables", bufs=1) as tbl,
                tc.tile_pool(name="xstage", bufs=4) as xstage,
                tc.tile_pool(name="xbstage", bufs=3) as xbstage,
                tc.tile_pool(name="xt", bufs=20) as xtpool,
                tc.tile_pool(name="rope", bufs=2) as rope,
                tc.tile_pool(name="vT", bufs=1) as vTpool,
                tc.tile_pool(name="qkvps", bufs=6, space="PSUM") as qkvps,
                tc.tile_pool(name="tps", bufs=2, space="PSUM") as tps,
            ):
                vT = [vTpool.tile([128, T], BF16, tag=f"vT{h}", name=f"vT{h}") for h in range(HL)]
                wsb = {}
                for nm, wt in (("q", wq_t), ("k", wk_t), ("v", wv_t)):
                    for d in range(NDT):
                        wtile = wpool.tile([128, EL], BF16, tag=f"w{nm}{d}", name=f"w{nm}{d}")
                        nc.gpsimd.dma_start(wtile[:], wt[d * 128:(d + 1) * 128, :])
                        wsb[(nm, d)] = wtile
                cos_sb = tbl.tile([128, S], BF16, tag="cos", name="cos_sb")
                nc.gpsimd.dma_start(cos_sb[:], cos_t[:])
                sin_sb = tbl.tile([128, S], BF16, tag="sin", name="sin_sb")
                nc.gpsimd.dma_start(sin_sb[:], sin_m[:])


                TG2 = 1024
                for tg2 in range(T // TG2):   # 1024-token groups
                    g0 = tg2 * TG2
                    # stage x: load f32, cast to bf16, store to DRAM scratch
                    for tt in range(TG2 // 128):
                        r0 = g0 + tt * 128
                        xf = xstage.tile([128, D], F32, tag="xf", name="xf")
                        nc.gpsimd.dma_start(xf[:], x[r0:r0 + 128, :])
                        xb = xbstage.tile([128, D], BF16, tag="xb", name="xb")
                        nc.scalar.add(xb[:], xf[:], 0.0)
                        nc.scalar.dma_start(
                            xbf[tg2][tt * 128:(tt + 1) * 128, :], xb[:])
                    # transpose group into SBUF: xt[d] = xbf[:, d*128:+128].T
                    xts = []
                    for d in range(NDT):
                        xtile = xtpool.tile([128, TG2], BF16, tag="xt", name="xt")
                        nc.sync.dma_start(
                            xtile[:], xbf[tg2][:, d * 128:(d + 1) * 128],
                            transpose=True)
                        xts.append(xtile)
                    # QKV matmuls per 512-token half (d-outer: frees xt fast)
                    for half in range(2):
                        t0 = g0 + half * TCH
                        hs = half * TCH
                        scol = ((t0 // TCH) % NQC) * TCH
                        pss = {}
                        for nm in ("q", "k", "v"):
                            for eh in range(HL):
                                pss[(nm, eh)] = qkvps.tile(
                                    [128, TCH], F32, tag="qkvps", name="qkvps")
                        for d in range(NDT):
                            for nm in ("q", "k", "v"):
                                for eh in range(HL):
                                    nc.tensor.matmul(
                                        pss[(nm, eh)][:],
                                        wsb[(nm, d)][:, eh * 128:(eh + 1) * 128],
                                        xts[d][:, hs:hs + TCH],
                                        start=(d == 0), stop=(d == NDT - 1))
                        for nm in ("q", "k", "v"):
                            for eh in range(HL):
                                ps = pss[(nm, eh)]
                                if nm == "v":
                                    nc.vector.tensor_copy(
                                        vT[eh][:, t0:t0 + TCH], ps[:])
                                else:
                                    dst = qT[eh] if nm == "q" else kT[eh]
                                    tmp = rope.tile([128, TCH], F32, tag="ropetmp", name="ropetmp")
                                    nc.vector.tensor_tensor(
                                        tmp[:], ps[:], cos_sb[:, scol:scol + TCH],
                                        OP.mult)
                                    u = rope.tile([128, TCH], F32, tag="ropeu", name="ropeu")
                                    nc.vector.tensor_tensor(
                                        u[0:64, :], ps[64:128, :],
                                        sin_sb[0:64, scol:scol + TCH], OP.mult)
                                    nc.vector.tensor_tensor(
                                        u[64:128, :], ps[0:64, :],
                                        sin_sb[64:128, scol:scol + TCH], OP.mult)
                                    nc.vector.tensor_tensor(
                                        dst[:, t0:t0 + TCH], tmp[:], u[:], OP.add)
                # batched v transposes to natural [t, hd] layout
                for eh in range(HL):
                    for ktg in range(T // 128):
                        c0 = ktg * 128
                        tp = tps.tile([128, 128], BF16, tag="tps", name="tpsum")
                        nc.tensor.transpose(
                            tp[:], vT[eh][:, c0:c0 + 128], ident_sb[:])
                        nc.vector.tensor_copy(vnat[eh][:, c0:c0 + 128], tp[:])
                if _DEBUG:
                    nc.sync.dma_start(dbg["qT0"][:], qT[0][:])
                    nc.sync.dma_start(dbg["kT0"][:], kT[0][:])
                    nc.sync.dma_start(dbg["vn0"][:], vnat[0][:])

            with tc.tile_pool(name="wo", bufs=1) as wopool:
                wo_sb = []
                for d in range(NDT):
                    wtile = wopool.tile([128, D], BF16, tag=f"wo{d}", name=f"wo{d}")
                    nc.gpsimd.dma_start(wtile[:], wo_t[d * 128:(d + 1) * 128, :])
                    wo_sb.append(wtile)
                # ---------- phase C: SDPA per (batch, head) ----------
                otpool = tc.alloc_tile_pool(name="ot", bufs=1)
                ot_sb = []
                with (
                    tc.tile_pool(name="E", bufs=NKT + 2) as epool,
                    tc.tile_pool(name="onorm", bufs=4) as onpool,
                    tc.tile_pool(name="rec", bufs=4) as recpool,
                    tc.tile_pool(name="sps", bufs=2, space="PSUM") as spool,
                    tc.tile_pool(name="ops", bufs=2, space="PSUM") as opool,
                    tc.tile_pool(name="dps", bufs=2, space="PSUM") as dpool,
                ):
                    for h in range(HL):
                        for b in range(B):
                            q0 = b * S
                            for qp in range(2):     # qt half: 1024 queries
                                qb = q0 + qp * 1024
                                E = []
                                ops, dps = [], []
                                for qc2 in range(2):
                                    ops.append(opool.tile([128, 512], F32, tag="ops", name="opsum"))
                                    dps.append(dpool.tile([128, 512], F32, tag="dps", name="dpsum"))
                                def attn_step(kt):
                                    e_t = E[kt]
                                    vsl = vnat[h][:, (b * NKT + kt) * 128:(b * NKT + kt + 1) * 128]
                                    for qc2 in range(2):
                                        erhs = e_t[:, qc2 * 512:(qc2 + 1) * 512]
                                        nc.tensor.matmul(
                                            ops[qc2][:], vsl, erhs,
                                            start=(kt == 0), stop=(kt == NKT - 1))
                                        nc.tensor.matmul(
                                            dps[qc2][:], ones_sb[:], erhs,
                                            start=(kt == 0), stop=(kt == NKT - 1))
                                for kt in range(NKT):
                                    sp = spool.tile([128, 1024], F32, tag="sps", name="spsum")
                                    for qh in range(2):
                                        nc.tensor.matmul(
                                            sp[:, qh * 512:(qh + 1) * 512],
                                            kT[h][:, q0 + kt * 128:q0 + (kt + 1) * 128],
                                            qT[h][:, qb + qh * 512:qb + (qh + 1) * 512],
                                            start=True, stop=True)
                                    e_t = epool.tile([128, 1024], BF16, tag="E", name="etile")
                                    mcol = b * NKT + kt
                                    nc.scalar.activation(
                                        e_t[:], sp[:], AF.Exp,
                                        bias=mask_sb[:, mcol:mcol + 1],
                                        scale=SCALE)
                                    E.append(e_t)
                                    if kt > 0:
                                        attn_step(kt - 1)
                                attn_step(NKT - 1)
                                for qc2 in range(2):
                                    qc = qp * 2 + qc2
                                    rec = recpool.tile([128, 512], F32, tag="rec", name="rec")
                                    nc.vector.reciprocal(rec[:], dps[qc2][:])
                                    on = onpool.tile([128, 512], BF16, tag="on", name="onorm")
                                    nc.vector.tensor_tensor(on[:], ops[qc2][:], rec[:], OP.mult)
                                    j = b * NQC + qc
                                    nc.gpsimd.dma_start(a2a_in[h][j, :, :], on[:])
                        # fire this head's AllToAll (head 0's overlaps head 1 SDPA)
                        nc.gpsimd.collective_compute(
                            "AllToAll", OP.bypass,
                            replica_groups=[list(range(NCORES))],
                            ins=[a2a_in[h].opt()],
                            outs=[a2a_out[h].opt()],
                        )
                    # OT loads: h0 tiles start right after the first AllToAll
                    for hh in range(HL):
                        for d in range(NDT):
                            if d % 2 != hh:
                                continue
                            otile = otpool.tile([128, TL], BF16, tag=f"ot{d}", name=f"ot{d}")
                            nc.sync.dma_start(
                                otile[:], a2a_out[d % 2][d // 2, :, :])
                            ot_sb.append((d, otile))
                    ot_sb = [t for _, t in sorted(ot_sb)]

                # ---------- phase D: output projection for my 512 tokens ----------
                with (
                    tc.tile_pool(name="ysb", bufs=8) as ypool,
                    tc.tile_pool(name="yps", bufs=8, space="PSUM") as ypsp,
                ):
                    for tt in range(TL // 128):
                        yps = [ypsp.tile([128, 512], F32, tag="yps", name="ypsum")
                               for _ in range(4)]
                        for d in range(NDT):
                            for eo in range(4):
                                nc.tensor.matmul(
                                    yps[eo][:],
                                    ot_sb[d][:, tt * 128:(tt + 1) * 128],
                                    wo_sb[d][:, eo * 512:(eo + 1) * 512],
                                    start=(d == 0), stop=(d == NDT - 1))
                        for eo in range(4):
                            ysb = ypool.tile([128, 512], F32, tag="ysb", name="ysb")
                            nc.vector.tensor_copy(ysb[:], yps[eo][:])
                            nc.gpsimd.dma_start(
                                out[tt * 128:(tt + 1) * 128,
                                    eo * 512:(eo + 1) * 512], ysb[:])
                otpool.release()

    nc.compile()
    return nc


def _prep_in_maps(x, cos, sin, attn_mask, wq, wk, wv, wo):
    xf = np.ascontiguousarray(x.reshape(T, D).astype(np.float32, copy=False))
    cosT = np.ascontiguousarray(np.asarray(cos[0], np.float32).T)   # [HD, S]
    sinT = np.asarray(sin[0], np.float32).T
    sin_m = np.ascontiguousarray(
        np.concatenate([-sinT[:64], sinT[64:]], axis=0))            # [HD, S]
    mask_t = np.ascontiguousarray(
        np.asarray(attn_mask, np.float32).reshape(B * NKT, 128).T)  # [128, 32]
    wo_t = np.ascontiguousarray(np.asarray(wo, np.float32).T.astype(bf16))
    in_maps = []
    for i in range(NCORES):
        sl = slice(i * EL, (i + 1) * EL)
        in_maps.append({
            "x": xf,
            "wq_t": np.ascontiguousarray(np.asarray(wq, np.float32)[sl].T.astype(bf16)),
            "wk_t": np.ascontiguousarray(np.asarray(wk, np.float32)[sl].T.astype(bf16)),
            "wv_t": np.ascontiguousarray(np.asarray(wv, np.float32)[sl].T.astype(bf16)),
            "wo_t": wo_t,
            "cos_t": cosT.astype(bf16),
            "sin_m": sin_m.astype(bf16),
            "mask_t": mask_t,
        })
    return in_maps


def kernel(x, cos, sin, attn_mask, wq, wk, wv, wo, _trace=False):
    if "nc" not in _CACHE:
        _CACHE["nc"] = _build()
    nc = _CACHE["nc"]
    in_maps = _prep_in_maps(x, cos, sin, attn_mask, wq, wk, wv, wo)
    res = run_bass_kernel_spmd(nc, in_maps, core_ids=list(range(NCORES)),
                               trace=_trace)
    _CACHE["last_result"] = res
    y = np.concatenate([np.asarray(res.results[i]["out"], np.float32)
                        for i in range(NCORES)], axis=0)
    return y.reshape(B, S, D)



# revision 2
# speedup vs baseline: 1.4709x; 1.4709x over previous
"""Trainium2 Bass kernel for multi-head attention with RoPE (B=2, S=2048,
D=2048, H=16), distributed over 8 NeuronCores with head tensor-parallelism
and an AllToAll to switch to token-parallelism for the output projection.

kernel(**inputs) takes the full unsharded inputs (as produced by the
reference setup_inputs) and returns the full [2, 2048, 2048] f32 output.

Layout strategy: x is pre-transposed/cast to bf16 [D, T] on the host (same
spirit as the host-side weight transposes), so QKV matmuls stream straight
from SBUF xT tiles with no on-device staging. V is produced directly in
natural [t, hd] layout by swapping matmul operands. The output projection
is split into per-head halves so head-0's half overlaps the second
AllToAll.
"""
import numpy as np
import ml_dtypes
from concourse import bass, bacc, tile, mybir
from concourse.bass_utils import run_bass_kernel_spmd

bf16 = ml_dtypes.bfloat16
BF16 = mybir.dt.bfloat16
F32 = mybir.dt.float32
AF = mybir.ActivationFunctionType
OP = mybir.AluOpType

B, S, D, H = 2, 2048, 2048, 16
HD = 128                 # head dim
NCORES = 8
HL = H // NCORES         # heads per core = 2
EL = HL * HD             # local projection width = 256
T = B * S                # 4096 flattened tokens
NG = 4                   # 1024-token groups in QKV phase
TG = T // NG             # 1024
NKT = S // 128           # 16 key tiles per batch
NQC = S // 512           # 4 query chunks per batch
NDT = D // 128           # 16 contraction tiles
TL = T // NCORES         # 512 tokens per core after AllToAll
SCALE = float(1.0 / np.sqrt(128.0))

_CACHE = {}


def _build():
    nc = bacc.Bacc("TRN2", target_bir_lowering=False, num_devices=NCORES)

    x_t = nc.dram_tensor("x_t", [D, T], BF16, kind="ExternalInput")
    wq_t = nc.dram_tensor("wq_t", [128, NDT * EL], BF16, kind="ExternalInput")
    wk_t = nc.dram_tensor("wk_t", [128, NDT * EL], BF16, kind="ExternalInput")
    wv_t = nc.dram_tensor("wv_t", [128, NDT * EL], BF16, kind="ExternalInput")
    wo_t = nc.dram_tensor("wo_t", [128, NDT * D], BF16, kind="ExternalInput")
    cos_t = nc.dram_tensor("cos_t", [HD, S], BF16, kind="ExternalInput")
    sin_m = nc.dram_tensor("sin_m", [HD, S], BF16, kind="ExternalInput")
    mask_t = nc.dram_tensor("mask_t", [128, B * NKT], F32, kind="ExternalInput")
    out = nc.dram_tensor("out", [TL, D], F32, kind="ExternalOutput")

    ones_dram = nc.inline_tensor(np.ones((128, 128), dtype=bf16), name="ones")

    with tile.TileContext(nc) as tc:
        with (
            tc.tile_pool(name="dram", bufs=1, space="DRAM") as dram,
            tc.tile_pool(name="consts", bufs=1) as consts,
            tc.tile_pool(name="keep", bufs=1) as keep,
        ):
            a2a_in = [dram.tile([NCORES, HD, TL], BF16, tag=f"a2a_in{h}",
                                name=f"a2a_in{h}") for h in range(HL)]
            a2a_out = [dram.tile([NCORES, HD, TL], BF16, tag=f"a2a_out{h}",
                                 name=f"a2a_out{h}") for h in range(HL)]

            ones_sb = consts.tile([128, 128], BF16, tag="ones", name="ones_sb")
            nc.sync.dma_start(ones_sb[:], ones_dram[:])
            mask_sb = consts.tile([128, B * NKT], F32, tag="mask", name="mask_sb")
            nc.sync.dma_start(mask_sb[:], mask_t[:])
            cos_sb = consts.tile([128, S], BF16, tag="cos", name="cos_sb")
            nc.sync.dma_start(cos_sb[:], cos_t[:])
            sin_sb = consts.tile([128, S], BF16, tag="sin", name="sin_sb")
            nc.sync.dma_start(sin_sb[:], sin_m[:])

            # persistent per-head tensors: qT/kT in [hd, t]; v natural packed
            # per 128-token block as [t=128, (eh, hd)] along the free dim
            qT = [keep.tile([128, T], BF16, tag=f"qT{h}", name=f"qT{h}")
                  for h in range(HL)]
            kT = [keep.tile([128, T], BF16, tag=f"kT{h}", name=f"kT{h}")
                  for h in range(HL)]
            vnat = keep.tile([128, 2 * T], BF16, tag="vnat", name="vnat")
            wo_sb = keep.tile([128, NDT * D], BF16, tag="wo", name="wo_sb")

            # ---------- phase A+B: QKV projections + RoPE ----------
            with (
                tc.tile_pool(name="wsb", bufs=1) as wpool,
                tc.tile_pool(name="xt", bufs=24) as xtpool,
                tc.tile_pool(name="rope", bufs=3) as rope,
                tc.tile_pool(name="qkps", bufs=4, space="PSUM") as qkps,
                tc.tile_pool(name="vps", bufs=4, space="PSUM") as vps,
            ):
                wsb = {}
                for nm, wt in (("q", wq_t), ("k", wk_t), ("v", wv_t)):
                    wtile = wpool.tile([128, NDT * EL], BF16, tag=f"w{nm}",
                                       name=f"w{nm}")
                    nc.sync.dma_start(wtile[:], wt[:])
                    wsb[nm] = wtile

                for g in range(NG):
                    g0 = g * TG
                    xts = []
                    for dti in range(NDT):
                        xtile = xtpool.tile([128, TG], BF16, tag="xt", name="xt")
                        nc.sync.dma_start(
                            xtile[:], x_t[dti * 128:(dti + 1) * 128, g0:g0 + TG])
                        xts.append(xtile)
                    if g == 1:
                        # wo is only needed at the output projection; fetch it
                        # now that the startup-critical loads are in flight
                        nc.scalar.dma_start(wo_sb[:], wo_t[:])
                    for half in range(2):
                        t0 = g0 + half * 512
                        pos0 = t0 % S
                        for nm in ("q", "k"):
                            for eh in range(HL):
                                ps = qkps.tile([128, 512], F32, tag="qkps",
                                               name="qkps")
                                for dti in range(NDT):
                                    nc.tensor.matmul(
                                        ps[:],
                                        wsb[nm][:, dti * EL + eh * 128:
                                                dti * EL + (eh + 1) * 128],
                                        xts[dti][:, half * 512:(half + 1) * 512],
                                        start=(dti == 0), stop=(dti == NDT - 1))
                                dst = qT[eh] if nm == "q" else kT[eh]
                                tmp = rope.tile([128, 512], F32, tag="ropetmp",
                                                name="ropetmp")
                                nc.vector.tensor_tensor(
                                    tmp[:], ps[:], cos_sb[:, pos0:pos0 + 512],
                                    OP.mult)
                                u = rope.tile([128, 512], F32, tag="ropeu",
                                              name="ropeu")
                                nc.vector.tensor_tensor(
                                    u[0:64, :], ps[64:128, :],
                                    sin_sb[0:64, pos0:pos0 + 512], OP.mult)
                                nc.vector.tensor_tensor(
                                    u[64:128, :], ps[0:64, :],
                                    sin_sb[64:128, pos0:pos0 + 512], OP.mult)
                                nc.vector.tensor_tensor(
                                    dst[:, t0:t0 + 512], tmp[:], u[:], OP.add)
                    for tb in range(TG // 128):
                        t0 = g0 + tb * 128
                        ps = vps.tile([128, EL], F32, tag="vps", name="vps")
                        for dti in range(NDT):
                            nc.tensor.matmul(
                                ps[:],
                                xts[dti][:, tb * 128:(tb + 1) * 128],
                                wsb["v"][:, dti * EL:(dti + 1) * EL],
                                start=(dti == 0), stop=(dti == NDT - 1))
                        nc.vector.tensor_copy(
                            vnat[:, t0 * 2:t0 * 2 + EL], ps[:])

            # ---------- phase C: SDPA per (head, batch, query-chunk) ----------
            ot_sb = {}
            with (
                tc.tile_pool(name="E", bufs=6) as epool,
                tc.tile_pool(name="onorm", bufs=3) as onpool,
                tc.tile_pool(name="rec", bufs=3) as recpool,
                tc.tile_pool(name="ot", bufs=1) as otpool,
                tc.tile_pool(name="ysb", bufs=1) as ypool,
                tc.tile_pool(name="ysum", bufs=4) as ysumpool,
                tc.tile_pool(name="sps", bufs=3, space="PSUM") as spool,
                tc.tile_pool(name="ops", bufs=2, space="PSUM") as opool,
                tc.tile_pool(name="dps", bufs=2, space="PSUM") as dpool,
                tc.tile_pool(name="yps", bufs=1, space="PSUM") as yppool,
            ):
                for h in range(HL):
                    for b in range(B):
                        q0 = b * S
                        for qc in range(NQC):
                            qb = q0 + qc * 512
                            ops_ps = opool.tile([128, 512], F32, tag="ops",
                                                name="opsum")
                            dps_ps = dpool.tile([128, 512], F32, tag="dps",
                                                name="dpsum")
                            E = []

                            def attn_step(kt):
                                e_t = E[kt]
                                vcol = (b * NKT + kt) * EL + h * 128
                                nc.tensor.matmul(
                                    ops_ps[:], vnat[:, vcol:vcol + 128], e_t[:],
                                    start=(kt == 0), stop=(kt == NKT - 1))
                                nc.tensor.matmul(
                                    dps_ps[:], ones_sb[:], e_t[:],
                                    start=(kt == 0), stop=(kt == NKT - 1))

                            for kt in range(NKT):
                                sp = spool.tile([128, 512], F32, tag="sps",
                                                name="spsum")
                                nc.tensor.matmul(
                                    sp[:],
                                    kT[h][:, q0 + kt * 128:q0 + (kt + 1) * 128],
                                    qT[h][:, qb:qb + 512],
                                    start=True, stop=True)
                                e_t = epool.tile([128, 512], BF16, tag="E",
                                                 name="etile")
                                mcol = b * NKT + kt
                                nc.scalar.activation(
                                    e_t[:], sp[:], AF.Exp,
                                    bias=mask_sb[:, mcol:mcol + 1],
                                    scale=SCALE)
                                E.append(e_t)
                                if kt > 0:
                                    attn_step(kt - 1)
                            attn_step(NKT - 1)

                            rec = recpool.tile([128, 512], F32, tag="rec",
                                               name="rec")
                            nc.vector.reciprocal_approx_fast(rec[:], dps_ps[:])
                            on = onpool.tile([128, 512], BF16, tag="on",
                                             name="onorm")
                            nc.vector.tensor_tensor(on[:], ops_ps[:], rec[:],
                                                    OP.mult)
                            j = b * NQC + qc
                            nc.sync.dma_start(a2a_in[h][j, :, :], on[:])
                    nc.gpsimd.collective_compute(
                        "AllToAll", OP.bypass,
                        replica_groups=[list(range(NCORES))],
                        ins=[a2a_in[h].opt()],
                        outs=[a2a_out[h].opt()],
                    )

                # ---------- phase D: output projection, split per head ----------
                # head-0's half runs while head-1's AllToAll is in flight
                for hh in range(HL):
                    for src in range(NCORES):
                        d = src * HL + hh
                        otile = otpool.tile([128, TL], BF16, tag=f"ot{d}",
                                            name=f"ot{d}")
                        nc.sync.dma_start(otile[:], a2a_out[hh][src, :, :])
                        ot_sb[d] = otile
                    for tt in range(TL // 128):
                        for eo in range(4):
                            yp = yppool.tile([128, 512], F32, tag="yps",
                                             name="ypsum")
                            for di in range(NCORES):
                                d = di * HL + hh
                                nc.tensor.matmul(
                                    yp[:],
                                    ot_sb[d][:, tt * 128:(tt + 1) * 128],
                                    wo_sb[:, d * D + eo * 512:
                                          d * D + (eo + 1) * 512],
                                    start=(di == 0), stop=(di == NCORES - 1))
                            if hh == 0:
                                y0 = ypool.tile([128, 512], F32,
                                                tag=f"y0_{tt}_{eo}",
                                                name=f"y0_{tt}_{eo}")
                                nc.vector.tensor_copy(y0[:], yp[:])
                                ot_sb[(0, tt, eo)] = y0
                            else:
                                ys = ysumpool.tile([128, 512], F32, tag="ysum",
                                                   name="ysum")
                                nc.vector.tensor_tensor(
                                    ys[:], yp[:], ot_sb[(0, tt, eo)][:], OP.add)
                                nc.sync.dma_start(
                                    out[tt * 128:(tt + 1) * 128,
                                        eo * 512:(eo + 1) * 512], ys[:])

    nc.compile()
    return nc


def _prep_in_maps(x, cos, sin, attn_mask, wq, wk, wv, wo):
    x_t = np.ascontiguousarray(
        np.asarray(x, np.float32).reshape(T, D).T.astype(bf16))      # [D, T]
    cosT = np.ascontiguousarray(np.asarray(cos[0], np.float32).T)    # [HD, S]
    sinT = np.asarray(sin[0], np.float32).T
    sin_m = np.ascontiguousarray(
        np.concatenate([-sinT[:64], sinT[64:]], axis=0))             # [HD, S]
    mask_t = np.ascontiguousarray(
        np.asarray(attn_mask, np.float32).reshape(B * NKT, 128).T)   # [128, 32]

    def pack(w_sl):
        # [E_out, D] slice -> [128, NDT * E_out] d-tile-major layout
        e_out = w_sl.shape[0]
        return np.ascontiguousarray(
            w_sl.T.reshape(NDT, 128, e_out).transpose(1, 0, 2)
            .reshape(128, NDT * e_out).astype(bf16))

    wo_t = pack(np.asarray(wo, np.float32))
    in_maps = []
    for i in range(NCORES):
        sl = slice(i * EL, (i + 1) * EL)
        in_maps.append({
            "x_t": x_t,
            "wq_t": pack(np.asarray(wq, np.float32)[sl]),
            "wk_t": pack(np.asarray(wk, np.float32)[sl]),
            "wv_t": pack(np.asarray(wv, np.float32)[sl]),
            "wo_t": wo_t,
            "cos_t": cosT.astype(bf16),
            "sin_m": sin_m.astype(bf16),
            "mask_t": mask_t,
        })
    return in_maps


def kernel(x, cos, sin, attn_mask, wq, wk, wv, wo, _trace=False):
    if "nc" not in _CACHE:
        _CACHE["nc"] = _build()
    nc = _CACHE["nc"]
    in_maps = _prep_in_maps(x, cos, sin, attn_mask, wq, wk, wv, wo)
    res = run_bass_kernel_spmd(nc, in_maps, core_ids=list(range(NCORES)),
                               trace=_trace)
    _CACHE["last_result"] = res
    y = np.concatenate([np.asarray(res.results[i]["out"], np.float32)
                        for i in range(NCORES)], axis=0)
    return y.reshape(B, S, D)


# revision 4
# speedup vs baseline: 1.5613x; 1.0615x over previous
"""Trainium2 Bass kernel for multi-head attention with RoPE (B=2, S=2048,
D=2048, H=16), distributed over 8 NeuronCores with head tensor-parallelism
and an AllToAll to switch to token-parallelism for the output projection.

kernel(**inputs) takes the full unsharded inputs (as produced by the
reference setup_inputs) and returns the full [2, 2048, 2048] f32 output.

Layout strategy: x is pre-transposed/cast to bf16 [D, T] on the host (same
spirit as the host-side weight transposes), so QKV matmuls stream straight
from SBUF xT tiles with no on-device staging. V is produced directly in
natural [t, hd] layout by swapping matmul operands. The output projection
is split into per-head halves so head-0's half overlaps the second
AllToAll.
"""
import numpy as np
import ml_dtypes
from concourse import bass, bacc, tile, mybir
from concourse.bass_utils import run_bass_kernel_spmd

bf16 = ml_dtypes.bfloat16
BF16 = mybir.dt.bfloat16
F32 = mybir.dt.float32
AF = mybir.ActivationFunctionType
OP = mybir.AluOpType

B, S, D, H = 2, 2048, 2048, 16
HD = 128                 # head dim
NCORES = 8
HL = H // NCORES         # heads per core = 2
EL = HL * HD             # local projection width = 256
T = B * S                # 4096 flattened tokens
NG = 4                   # 1024-token groups in QKV phase
TG = T // NG             # 1024
NKT = S // 128           # 16 key tiles per batch
NQC = S // 512           # 4 query chunks per batch
NDT = D // 128           # 16 contraction tiles
TL = T // NCORES         # 512 tokens per core after AllToAll
SCALE = float(1.0 / np.sqrt(128.0))

_CACHE = {}


def _build():
    nc = bacc.Bacc("TRN2", target_bir_lowering=False, num_devices=NCORES)

    x_t = nc.dram_tensor("x_t", [D, T], BF16, kind="ExternalInput")
    wq_t = nc.dram_tensor("wq_t", [128, NDT * EL], BF16, kind="ExternalInput")
    wk_t = nc.dram_tensor("wk_t", [128, NDT * EL], BF16, kind="ExternalInput")
    wv_t = nc.dram_tensor("wv_t", [128, NDT * EL], BF16, kind="ExternalInput")
    wo_t = nc.dram_tensor("wo_t", [128, NDT * D], BF16, kind="ExternalInput")
    cos_t = nc.dram_tensor("cos_t", [HD, S], BF16, kind="ExternalInput")
    sin_m = nc.dram_tensor("sin_m", [HD, S], BF16, kind="ExternalInput")
    mask_t = nc.dram_tensor("mask_t", [128, B * NKT], F32, kind="ExternalInput")
    out = nc.dram_tensor("out", [TL, D], F32, kind="ExternalOutput")

    ones_dram = nc.inline_tensor(np.ones((128, 128), dtype=bf16), name="ones")

    with tile.TileContext(nc) as tc:
        with (
            tc.tile_pool(name="dram", bufs=1, space="DRAM") as dram,
            tc.tile_pool(name="consts", bufs=1) as consts,
            tc.tile_pool(name="keep", bufs=1) as keep,
        ):
            a2a_in = [dram.tile([NCORES, HD, TL], BF16, tag=f"a2a_in{h}",
                                name=f"a2a_in{h}") for h in range(HL)]
            a2a_out = [dram.tile([NCORES, HD, TL], BF16, tag=f"a2a_out{h}",
                                 name=f"a2a_out{h}") for h in range(HL)]

            ones_sb = consts.tile([128, 128], BF16, tag="ones", name="ones_sb")
            nc.sync.dma_start(ones_sb[:], ones_dram[:])
            mask_sb = consts.tile([128, B * NKT], F32, tag="mask", name="mask_sb")
            nc.sync.dma_start(mask_sb[:], mask_t[:])
            cos_sb = consts.tile([128, S], BF16, tag="cos", name="cos_sb")
            nc.sync.dma_start(cos_sb[:], cos_t[:])
            sin_sb = consts.tile([128, S], BF16, tag="sin", name="sin_sb")
            nc.sync.dma_start(sin_sb[:], sin_m[:])

            # persistent per-head tensors: qT/kT in [hd, t]; v natural packed
            # per 128-token block as [t=128, (eh, hd)] along the free dim
            qT = [keep.tile([128, T], BF16, tag=f"qT{h}", name=f"qT{h}")
                  for h in range(HL)]
            kT = [keep.tile([128, T], BF16, tag=f"kT{h}", name=f"kT{h}")
                  for h in range(HL)]
            vnat = keep.tile([128, 2 * T], BF16, tag="vnat", name="vnat")
            wo_sb = keep.tile([128, NDT * D], BF16, tag="wo", name="wo_sb")

            # ---------- phase A+B: QKV projections + RoPE ----------
            with (
                tc.tile_pool(name="wsb", bufs=1) as wpool,
                tc.tile_pool(name="xt", bufs=24) as xtpool,
                tc.tile_pool(name="rope", bufs=3) as rope,
                tc.tile_pool(name="qkps", bufs=4, space="PSUM") as qkps,
                tc.tile_pool(name="vps", bufs=4, space="PSUM") as vps,
            ):
                wsb = {}
                for nm, wt in (("q", wq_t), ("k", wk_t), ("v", wv_t)):
                    wtile = wpool.tile([128, NDT * EL], BF16, tag=f"w{nm}",
                                       name=f"w{nm}")
                    nc.sync.dma_start(wtile[:], wt[:])
                    wsb[nm] = wtile

                for g in range(NG):
                    g0 = g * TG
                    xts = []
                    for dti in range(NDT):
                        xtile = xtpool.tile([128, TG], BF16, tag="xt", name="xt")
                        nc.sync.dma_start(
                            xtile[:], x_t[dti * 128:(dti + 1) * 128, g0:g0 + TG])
                        xts.append(xtile)
                    if g == 2:
                        # wo is only needed at the output projection; fetch it
                        # once the startup-critical loads are done draining
                        nc.scalar.dma_start(wo_sb[:], wo_t[:])
                    for half in range(2):
                        t0 = g0 + half * 512
                        pos0 = t0 % S
                        for nm in ("q", "k"):
                            for eh in range(HL):
                                ps = qkps.tile([128, 512], F32, tag="qkps",
                                               name="qkps")
                                for dti in range(NDT):
                                    nc.tensor.matmul(
                                        ps[:],
                                        wsb[nm][:, dti * EL + eh * 128:
                                                dti * EL + (eh + 1) * 128],
                                        xts[dti][:, half * 512:(half + 1) * 512],
                                        start=(dti == 0), stop=(dti == NDT - 1))
                                dst = qT[eh] if nm == "q" else kT[eh]
                                tmp = rope.tile([128, 512], F32, tag="ropetmp",
                                                name="ropetmp")
                                nc.vector.tensor_tensor(
                                    tmp[:], ps[:], cos_sb[:, pos0:pos0 + 512],
                                    OP.mult)
                                u = rope.tile([128, 512], F32, tag="ropeu",
                                              name="ropeu")
                                nc.vector.tensor_tensor(
                                    u[0:64, :], ps[64:128, :],
                                    sin_sb[0:64, pos0:pos0 + 512], OP.mult)
                                nc.vector.tensor_tensor(
                                    u[64:128, :], ps[0:64, :],
                                    sin_sb[64:128, pos0:pos0 + 512], OP.mult)
                                nc.vector.tensor_tensor(
                                    dst[:, t0:t0 + 512], tmp[:], u[:], OP.add)
                    for tb in range(TG // 128):
                        t0 = g0 + tb * 128
                        ps = vps.tile([128, EL], F32, tag="vps", name="vps")
                        for dti in range(NDT):
                            nc.tensor.matmul(
                                ps[:],
                                xts[dti][:, tb * 128:(tb + 1) * 128],
                                wsb["v"][:, dti * EL:(dti + 1) * EL],
                                start=(dti == 0), stop=(dti == NDT - 1))
                        nc.vector.tensor_copy(
                            vnat[:, t0 * 2:t0 * 2 + EL], ps[:])

            # ---------- phase C: SDPA per (head, batch, 1024-query block) ----------
            ot_sb = {}
            with (
                tc.tile_pool(name="E", bufs=6) as epool,
                tc.tile_pool(name="Epair", bufs=3) as eppool,
                tc.tile_pool(name="onorm", bufs=3) as onpool,
                tc.tile_pool(name="rec", bufs=3) as recpool,
                tc.tile_pool(name="sps", bufs=2, space="PSUM") as spool,
                tc.tile_pool(name="ops", bufs=2, space="PSUM") as opool,
                tc.tile_pool(name="dps", bufs=2, space="PSUM") as dpool,
            ):
                for h in range(HL):
                    for b in range(B):
                        q0 = b * S
                        for qp in range(2):
                            qb = q0 + qp * 1024
                            ops_ps = [opool.tile([128, 512], F32, tag="ops",
                                                 name="opsum")
                                      for _ in range(2)]
                            dps_ps = [dpool.tile([128, 512], F32, tag="dps",
                                                 name="dpsum")
                                      for _ in range(2)]
                            E = []

                            def attn_step(kt):
                                e_t = E[kt]
                                vcol = (b * NKT + kt) * EL + h * 128
                                for qc2 in range(2):
                                    nc.tensor.matmul(
                                        ops_ps[qc2][:],
                                        vnat[:, vcol:vcol + 128],
                                        e_t[:, qc2 * 512:(qc2 + 1) * 512],
                                        start=(kt == 0), stop=(kt == NKT - 1))
                                if kt % 2 == 1:
                                    ep = eppool.tile([128, 1024], BF16,
                                                     tag="epair", name="epair")
                                    nc.vector.tensor_tensor(
                                        ep[:], E[kt - 1][:], e_t[:], OP.add)
                                    for qc2 in range(2):
                                        nc.tensor.matmul(
                                            dps_ps[qc2][:], ones_sb[:],
                                            ep[:, qc2 * 512:(qc2 + 1) * 512],
                                            start=(kt == 1),
                                            stop=(kt == NKT - 1))

                            for kt in range(NKT):
                                sp = spool.tile([128, 1024], F32, tag="sps",
                                                name="spsum")
                                for qc2 in range(2):
                                    nc.tensor.matmul(
                                        sp[:, qc2 * 512:(qc2 + 1) * 512],
                                        kT[h][:, q0 + kt * 128:
                                              q0 + (kt + 1) * 128],
                                        qT[h][:, qb + qc2 * 512:
                                              qb + (qc2 + 1) * 512],
                                        start=True, stop=True)
                                e_t = epool.tile([128, 1024], BF16, tag="E",
                                                 name="etile")
                                mcol = b * NKT + kt
                                nc.scalar.activation(
                                    e_t[:], sp[:], AF.Exp,
                                    bias=mask_sb[:, mcol:mcol + 1],
                                    scale=SCALE)
                                E.append(e_t)
                                if kt > 0:
                                    attn_step(kt - 1)
                            attn_step(NKT - 1)

                            for qc2 in range(2):
                                rec = recpool.tile([128, 512], F32, tag="rec",
                                                   name="rec")
                                nc.vector.reciprocal_approx_fast(
                                    rec[:], dps_ps[qc2][:])
                                on = onpool.tile([128, 512], BF16, tag="on",
                                                 name="onorm")
                                nc.vector.tensor_tensor(
                                    on[:], ops_ps[qc2][:], rec[:], OP.mult)
                                j = b * NQC + qp * 2 + qc2
                                nc.sync.dma_start(a2a_in[h][j, :, :], on[:])
                    nc.gpsimd.collective_compute(
                        "AllToAll", OP.bypass,
                        replica_groups=[list(range(NCORES))],
                        ins=[a2a_in[h].opt()],
                        outs=[a2a_out[h].opt()],
                    )

            # ---------- phase D: output projection, split per head ----------
            # head-0's half runs while head-1's AllToAll is in flight
            with (
                tc.tile_pool(name="ot", bufs=1) as otpool,
                tc.tile_pool(name="ysb", bufs=1) as ypool,
                tc.tile_pool(name="ysum", bufs=4) as ysumpool,
                tc.tile_pool(name="yps", bufs=2, space="PSUM") as yppool,
            ):
                for hh in range(HL):
                    for src in range(NCORES):
                        d = src * HL + hh
                        otile = otpool.tile([128, TL], BF16, tag=f"ot{d}",
                                            name=f"ot{d}")
                        nc.sync.dma_start(otile[:], a2a_out[hh][src, :, :])
                        ot_sb[d] = otile
                    for tt in range(TL // 128):
                        for eo in range(4):
                            yp = yppool.tile([128, 512], F32, tag="yps",
                                             name="ypsum")
                            for di in range(NCORES):
                                d = di * HL + hh
                                nc.tensor.matmul(
                                    yp[:],
                                    ot_sb[d][:, tt * 128:(tt + 1) * 128],
                                    wo_sb[:, d * D + eo * 512:
                                          d * D + (eo + 1) * 512],
                                    start=(di == 0), stop=(di == NCORES - 1))
                            if hh == 0:
                                y0 = ypool.tile([128, 512], F32,
                                                tag=f"y0_{tt}_{eo}",
                                                name=f"y0_{tt}_{eo}")
                                nc.vector.tensor_copy(y0[:], yp[:])
                                ot_sb[(0, tt, eo)] = y0
                            else:
                                ys = ysumpool.tile([128, 512], F32, tag="ysum",
                                                   name="ysum")
                                nc.vector.tensor_tensor(
                                    ys[:], yp[:], ot_sb[(0, tt, eo)][:], OP.add)
                                nc.sync.dma_start(
                                    out[tt * 128:(tt + 1) * 128,
                                        eo * 512:(eo + 1) * 512], ys[:])

    nc.compile()
    return nc


def _prep_in_maps(x, cos, sin, attn_mask, wq, wk, wv, wo):
    x_t = np.ascontiguousarray(
        np.asarray(x, np.float32).reshape(T, D).T.astype(bf16))      # [D, T]
    cosT = np.ascontiguousarray(np.asarray(cos[0], np.float32).T)    # [HD, S]
    sinT = np.asarray(sin[0], np.float32).T
    sin_m = np.ascontiguousarray(
        np.concatenate([-sinT[:64], sinT[64:]], axis=0))             # [HD, S]
    mask_t = np.ascontiguousarray(
        np.asarray(attn_mask, np.float32).reshape(B * NKT, 128).T)   # [128, 32]

    def pack(w_sl):
        # [E_out, D] slice -> [128, NDT * E_out] d-tile-major layout
        e_out = w_sl.shape[0]
        return np.ascontiguousarray(
            w_sl.T.reshape(NDT, 128, e_out).transpose(1, 0, 2)
            .reshape(128, NDT * e_out).astype(bf16))

    wo_t = pack(np.asarray(wo, np.float32))
    in_maps = []
    for i in range(NCORES):
        sl = slice(i * EL, (i + 1) * EL)
        in_maps.append({
            "x_t": x_t,
            "wq_t": pack(np.asarray(wq, np.float32)[sl]),
            "wk_t": pack(np.asarray(wk, np.float32)[sl]),
            "wv_t": pack(np.asarray(wv, np.float32)[sl]),
            "wo_t": wo_t,
            "cos_t": cosT.astype(bf16),
            "sin_m": sin_m.astype(bf16),
            "mask_t": mask_t,
        })
    return in_maps


def kernel(x, cos, sin, attn_mask, wq, wk, wv, wo, _trace=False):
    if "nc" not in _CACHE:
        _CACHE["nc"] = _build()
    nc = _CACHE["nc"]
    in_maps = _prep_in_maps(x, cos, sin, attn_mask, wq, wk, wv, wo)
    res = run_bass_kernel_spmd(nc, in_maps, core_ids=list(range(NCORES)),
                               trace=_trace)
    _CACHE["last_result"] = res
    y = np.concatenate([np.asarray(res.results[i]["out"], np.float32)
                        for i in range(NCORES)], axis=0)
    return y.reshape(B, S, D)
